# revision 63
# baseline (speedup 1.0000x reference)
"""Causal self-attention (d_model=1024, n_head=16, seq=4096) on 8 trn2 cores.

Sharding: tensor-parallel over heads (2 heads/core) for QKV + attention,
then an AllToAll re-shards y^T from head-sharded to sequence-sharded, so
each core runs the output projection for seq/8 rows with the full w_proj
(no AllReduce). The host concatenates the 8 row-shards.

v2 layout (vs the earlier baseline):
  - x^T and all weights are pre-transposed/bf16-converted on the host as
    part of sharding, so the device program has no PE-transpose or
    fp32->bf16 staging phase at all: DMAs land directly in matmul layout.
  - qkv^T = w_slice.T @ x^T gives qT/kT in [chan, T]; V is computed in
    normal [T, chan] orientation (lhsT = x^T tile) and packed per k-tile
    as [v_h|1] columns for the denominator trick.
  - attention per q-block of 512: scores^T = K Q^T per k-tile (PSUM),
    exp on ACT straight into bf16 SBUF, diagonal tiles masked by a
    precomputed 0/1 multiply, then y = P V computed in [q, d] orientation
    (lhsT = exp-probs slice, N=65 per matmul) accumulating over k-tiles.
    This costs ~2x less PE time than the [d, q] orientation because the
    per-accumulation-step output is 65 cols instead of 512.
  - softmax normalization on DVE: reciprocal of the ones-column then a
    per-partition tensor_scalar multiply (no Ln/Exp or broadcast matmul).
  - normalized y tiles are PE-transposed back to [chan, T] for the
    AllToAll staging, off the critical path (braided into next block).
  - emission is braided: prep work for later blocks sits in a global
    queue drained at a uniform rate across all 72 score groups (x^T DMAs
    prefetched ~2 blocks ahead), and av(g) is emitted after
    scores_exp(g+1) so the PE never waits in-order on an in-flight exp.
  - PE p-state management: junk warm-up matmuls at t=0 and a keep-warm
    matmul stream spanning the AllToAll + receive window, so the qkv
    prologue and the output projection both dispatch at full clock.
"""

import sys
import types

import numpy as np
import ml_dtypes

D_MODEL = 1024
N_HEAD = 16
SEQ = 4096
N_CORES = 8
D_HEAD = 64
CPC = 128            # channels per core (2 heads x 64)
QB = 512             # attention q-block width
BF16 = ml_dtypes.bfloat16


def _install_compat_patches():
    """Stub antenv.axon_hooks (absent in this container) so
    run_bass_kernel_spmd's trace path degrades instead of ImportError."""
    if "antenv.axon_hooks" not in sys.modules:
        mod = types.ModuleType("antenv.axon_hooks")
        mod.get_axon_ntff_profile_hook = lambda: None
        sys.modules["antenv.axon_hooks"] = mod


def _split_multi_waits(nc):
    """The nix walrus here accepts at most ONE sync-wait per instruction
    (setupSyncWait: 'Too many sync wait commands').  Hoist extra waits onto
    same-engine NoOps inserted immediately before the instruction — engine
    streams execute in program order, so semantics are unchanged."""
    import concourse.mybir as mybir

    n = 0
    for fn in nc.m.functions:
        for bb in fn.blocks:
            insts = bb.instructions
            out = []
            for inst in insts:
                si = getattr(inst, "sync_info", None)
                waits = list(si.on_wait) if si is not None else []
                if len(waits) > 1:
                    si.on_wait.clear()
                    for w in waits[:-1]:
                        n += 1
                        nop = mybir.InstNoOp(name=f"I-WSPLIT{n}", ins=[], outs=[])
                        nop.engine = inst.engine
                        nop.sync_info = mybir.SyncInfo(on_wait=[w], on_update=[])
                        out.append(nop)
                    si.on_wait.append(waits[-1])
                out.append(inst)
            bb.instructions = out


def build_nc(seq=SEQ, use_collective=True, split_waits=True, debug=False):
    """Build the single-core SPMD program (identical on all 8 cores)."""
    import concourse.bass as bass
    import concourse.mybir as mybir
    from concourse.tile import TileContext

    _install_compat_patches()

    f32 = mybir.dt.float32
    bf16 = mybir.dt.bfloat16
    AFT = mybir.ActivationFunctionType

    nT = seq // 128       # k-tiles
    nQB = seq // QB       # q-blocks
    SW = seq // N_CORES   # AllToAll shard width (output rows per core)

    nc = bass.Bass("TRN2", target_bir_lowering=False, debug=False,
                   num_devices=N_CORES)
    xt_d = nc.dram_tensor("xT", [D_MODEL, seq], bf16,
                          kind="ExternalInput").ap()
    wq_d = nc.dram_tensor("w_slice", [D_MODEL, 3 * CPC], bf16,
                          kind="ExternalInput").ap()
    wp_d = nc.dram_tensor("w_proj", [D_MODEL, D_MODEL], bf16,
                          kind="ExternalInput").ap()
    id_d = nc.dram_tensor("ident", [128, 128], bf16, kind="ExternalInput").ap()
    mk_d = nc.dram_tensor("masks", [4, 128, QB], bf16,
                          kind="ExternalInput").ap()
    out_d = nc.dram_tensor("out", [SW, D_MODEL], f32,
                           kind="ExternalOutput").ap()
    if debug:
        dbg = {
            nm: nc.dram_tensor(f"dbg_{nm}", shape, bf16,
                               kind="ExternalOutput").ap()
            for nm, shape in [
                ("qT", [128, seq]), ("kT", [128, seq]),
                ("V2", [128, (seq // 128) * 130]), ("ynT", [128, seq]),
                ("a2a", [N_CORES * CPC, SW]),
                ("pt0", [128, 4 * 2 * QB]), ("av0", [128, 2 * 4 * 65]),
            ]
        }

    with TileContext(nc) as tc:
        with (
            tc.tile_pool(name="per", bufs=1) as per,
            tc.tile_pool(name="stg", bufs=2) as stg,
            tc.tile_pool(name="dram", bufs=1, space="DRAM") as dram,
        ):
            xT = per.tile([128, 8, seq], bf16)   # [c-chunk part, chunk, T]
            wqkv = per.tile([128, 8, 3 * CPC], bf16)
            wpj = per.tile([128, 8, D_MODEL], bf16)
            qT = per.tile([128, seq], bf16)      # [2 heads x 64 d, T]
            kT = per.tile([128, seq], bf16)
            V2 = per.tile([128, nT, 130], bf16)  # per k-tile [v_h0|1|v_h1|1]
            ynT = per.tile([128, seq], bf16)     # normalized y^T (chan, T)
            mks = per.tile([128, 4, QB], bf16)
            iden = per.tile([128, 128], bf16)
            a2a_sb = per.tile([128, 8, SW], bf16)
            junk = per.tile([128, 128], bf16)   # never written: warm-up input

            xt_r = xt_d.rearrange("(k p) t -> p k t", k=8)
            wq_r = wq_d.rearrange("(k p) c -> p k c", k=8)
            wp_r = wp_d.rearrange("(k p) c -> p k c", k=8)
            mk_r = mk_d.rearrange("m p q -> p m q")

            nc.any.memset(junk[:], 1.0)
            nc.any.memset(V2[:, :, 64:65], 1.0)
            nc.any.memset(V2[:, :, 129:130], 1.0)

            a2a_in = dram.tile([N_CORES * CPC, SW], bf16)
            a2a_out = dram.tile([N_CORES * CPC, SW], bf16)

            # ---- braided attention phases --------------------------------
            # PSUM budget (8 banks): sc 2x[128,1024] = 4, av 2x[128,4x65]=2,
            # prep 2x[128,512] = 2.
            with (
                tc.tile_pool(name="ps", bufs=2, space="PSUM") as ps,
            ):



                def prep_chunks(n):
                    """Fine-grained closures for block n: x^T loads, qkv
                    matmuls, V pack.  Each chunk is <= ~450ns of PE so the
                    braid never delays the next score group much."""
                    c0, c1 = QB * n, QB * (n + 1)
                    state = {}

                    def loads():
                        nc.sync.dma_start(xT[:, :, c0:c1],
                                          xt_r[:, :, c0:c1])

                    def qk_mm(m, ks):
                        def emit():
                            if ks[0] == 0:
                                state[f"qp{m}"] = ps.tile(
                                    [128, QB], f32, tag="prep",
                                    name=f"qp_{n}_{m}")
                            qp = state[f"qp{m}"]
                            for k in ks:
                                nc.tensor.matmul(
                                    qp[:],
                                    wqkv[:, k, 128 * m:128 * (m + 1)],
                                    xT[:, k, c0:c1],
                                    start=(k == 0), stop=(k == 7))
                        return emit

                    def qk_cp(m):
                        def emit():
                            dst = qT if m == 0 else kT
                            nc.vector.tensor_copy(dst[:, c0:c1],
                                                  state[f"qp{m}"][:])
                        return emit

                    def v_mm(us):
                        def emit():
                            if us[0] == 0:
                                state["vp"] = ps.tile([128, QB], f32,
                                                      tag="prep",
                                                      name=f"vp_{n}")
                            vp = state["vp"]
                            for u in us:
                                t = 4 * n + u
                                for k in range(8):
                                    nc.tensor.matmul(
                                        vp[:, 128 * u:128 * (u + 1)],
                                        xT[:, k, 128 * t:128 * (t + 1)],
                                        wqkv[:, k, 2 * CPC:3 * CPC],
                                        start=(k == 0), stop=(k == 7))
                        return emit

                    def vcp(u):
                        def emit():
                            t = 4 * n + u
                            vp = state["vp"]
                            nc.vector.tensor_copy(
                                V2[:, t, 0:64], vp[:, 128 * u:128 * u + 64])
                            nc.vector.tensor_copy(
                                V2[:, t, 65:129],
                                vp[:, 128 * u + 64:128 * (u + 1)])
                        return emit

                    out = []
                    for m in (0, 1):
                        out += [qk_mm(m, (0, 1)), qk_mm(m, (2, 3)),
                                qk_mm(m, (4, 5)), qk_mm(m, (6, 7)),
                                qk_cp(m)]
                    out += [v_mm((0,)), v_mm((1,)), v_mm((2,)), v_mm((3,))]
                    out += [vcp(u) for u in range(4)]
                    return loads, out

                def attention_groups(n, avs):
                    """Returns (scores_exp, av) closure lists.  The caller
                    emits av(g) AFTER scores_exp(g+1) so the PE never sits
                    in-order behind an exp it is still waiting for."""
                    nkt = 4 * (n + 1)
                    c0 = QB * n
                    pts = {}

                    def off(kt):
                        d = kt - 4 * n
                        return 128 * d if d >= 0 else 0

                    def scores_exp(g):
                        def emit():
                            sps = []
                            for h in (0, 1):
                                sp = ps.tile([128, 2 * QB], f32, tag="sc",
                                             name=f"sp_{n}_{g}_{h}")
                                sps.append(sp)
                                for u in (0, 1):
                                    kt = 2 * g + u
                                    o = off(kt)
                                    nc.tensor.matmul(
                                        sp[:, QB * u + o:QB * (u + 1)],
                                        kT[64 * h:64 * (h + 1),
                                           128 * kt:128 * (kt + 1)],
                                        qT[64 * h:64 * (h + 1),
                                           c0 + o:c0 + QB],
                                        start=True, stop=True)
                            diag = off(2 * g) > 0 or off(2 * g + 1) > 0
                            for h in (0, 1):
                                pt = stg.tile([128, 2 * QB], bf16, tag="pt",
                                              bufs=4, name=f"pt_{n}_{g}_{h}")
                                pts[(g, h)] = pt
                                if diag:
                                    for u in (0, 1):
                                        o = off(2 * g + u)
                                        nc.scalar.activation(
                                            pt[:, QB * u + o:QB * (u + 1)],
                                            sps[h][:, QB * u + o:QB * (u + 1)],
                                            AFT.Exp, scale=0.125)
                                else:
                                    nc.scalar.activation(pt[:], sps[h][:],
                                                         AFT.Exp, scale=0.125)
                                for u in (0, 1):
                                    kt = 2 * g + u
                                    d = kt - 4 * n
                                    o = off(kt)
                                    if d >= 0:
                                        nc.vector.tensor_mul(
                                            pt[:, QB * u + o:QB * (u + 1)],
                                            pt[:, QB * u + o:QB * (u + 1)],
                                            mks[:, d, o:QB])
                                if debug and n == 0:
                                    nc.sync.dma_start(
                                        dbg["pt0"].rearrange(
                                            "p (g h q) -> p g h q",
                                            g=2, h=2)[:, g, h],
                                        pt[:])
                        return emit

                    def av(g):
                        # PSUM start_tensor_calc marks the whole 2KB zero
                        # region pending-zero, so interleaved accumulator
                        # chains in one bank must issue exactly ONE start:
                        # the first matmul of the tile.  Later first-touches
                        # of other slices overwrite via pending-zero.
                        def emit():
                            for h in (0, 1):
                                pt = pts.pop((g, h))
                                for u in (0, 1):
                                    kt = 2 * g + u
                                    d = kt - 4 * n
                                    for qt in range(max(d, 0), 4):
                                        nc.tensor.matmul(
                                            avs[h][:, qt, :],
                                            pt[:, QB * u + 128 * qt:
                                               QB * u + 128 * (qt + 1)],
                                            V2[:, kt, 65 * h:65 * (h + 1)],
                                            start=(kt == 0 and qt == 0),
                                            stop=(kt == 4 * n + 3
                                                  and qt == 3))
                        return emit

                    ng = nkt // 2
                    return ([scores_exp(g) for g in range(ng)],
                            [av(g) for g in range(ng)])

                def normalize(n, avs, qts=(0, 1, 2, 3)):
                    c0 = QB * n
                    if debug and n == 0:
                        for h in (0, 1):
                            dav = stg.tile([128, 4, 65], bf16, tag="dav",
                                           bufs=2, name=f"dav_{h}")
                            nc.vector.tensor_copy(dav[:], avs[h][:])
                            nc.sync.dma_start(
                                dbg["av0"].rearrange(
                                    "p (h t c) -> p h t c", h=2, t=4)[:, h],
                                dav[:])
                    q0, q1 = qts[0], qts[-1] + 1
                    rcs = []
                    for h in (0, 1):
                        dcp = stg.tile([128, 4], f32, tag="dcp", bufs=2,
                                       name=f"dcp_{n}_{h}_{q0}")
                        nc.vector.tensor_copy(dcp[:, q0:q1],
                                              avs[h][:, q0:q1, 64])
                        rc = stg.tile([128, 4], f32, tag="rc", bufs=2,
                                      name=f"rc_{n}_{h}_{q0}")
                        nc.vector.reciprocal(rc[:, q0:q1], dcp[:, q0:q1])
                        rcs.append(rc)
                    tp = ps.tile([128, QB], f32, tag="prep",
                                 name=f"tp_{n}_{q0}")
                    for qt in qts:
                        yn = stg.tile([128, 128], bf16, tag="yn", bufs=2,
                                      name=f"yn_{n}_{qt}")
                        for h in (0, 1):
                            nc.vector.tensor_scalar_mul(
                                yn[:, 64 * h:64 * (h + 1)],
                                avs[h][:, qt, 0:64],
                                rcs[h][:, qt:qt + 1])
                        nc.tensor.matmul(tp[:, 128 * qt:128 * (qt + 1)],
                                         yn[:], iden[:],
                                         start=True, stop=True)
                        nc.vector.tensor_copy(
                            ynT[:, c0 + 128 * qt:c0 + 128 * (qt + 1)],
                            tp[:, 128 * qt:128 * (qt + 1)])
                    nc.sync.dma_start(
                        a2a_in[CPC * n:CPC * (n + 1), 128 * q0:128 * q1],
                        ynT[:, c0 + 128 * q0:c0 + 128 * q1])

                # ---- emission ---------------------------------------
                # First loads split in halves so the first qkv matmuls can
                # start as soon as the first half lands.
                _, p0 = prep_chunks(0)
                nc.sync.dma_start(xT[:, 0:2, 0:QB], xt_r[:, 0:2, 0:QB])
                nc.sync.dma_start(wqkv[:, 0:2, :], wq_r[:, 0:2, :])
                nc.sync.dma_start(xT[:, 2:4, 0:QB], xt_r[:, 2:4, 0:QB])
                nc.sync.dma_start(wqkv[:, 2:4, :], wq_r[:, 2:4, :])
                nc.sync.dma_start(xT[:, 4:8, 0:QB], xt_r[:, 4:8, 0:QB])
                nc.sync.dma_start(wqkv[:, 4:8, :], wq_r[:, 4:8, :])
                nc.sync.dma_start(mks[:], mk_r)
                nc.sync.dma_start(iden[:], id_d[:])
                # preload the Exp activation table while ACT is idle
                dume = stg.tile([1, 8], f32, tag="dume", bufs=1)
                nc.any.memset(dume[:], 0.0)
                nc.scalar.activation(dume[:], dume[:], AFT.Exp, scale=1.0)
                # PE p-state warm-up: junk matmuls (uninitialized input - the
                # results are never read) from t=0 while the first x^T/w DMAs
                # are in flight, so the real qkv matmuls dispatch at full
                # clock.
                wp0 = ps.tile([128, QB], f32, tag="prep", name="warm0")
                for i in range(30):
                    nc.tensor.matmul(wp0[:, 0:128], junk[:], junk[:],
                                     start=True, stop=True)
                p0qk, p0v = p0[:10], p0[10:]
                for c in p0qk:
                    c()
                # Per-block emission, software-pipelined: av(g) emitted after
                # scores_exp(g+1), and the last av + normalize of block n are
                # deferred behind the first scores of block n+1 so the PE
                # never waits in-order on an exp still in flight.  Prep work
                # for later blocks sits in a single global queue, consumed at
                # a uniform rate across all 72 groups (with a hard deadline:
                # prep(k) fully emitted before block k's first scores).
                # x^T loads are issued ~2 blocks ahead of the matmuls that
                # consume them so the PE never waits on an in-flight DMA.
                queue = []            # (deadline_block, closure)
                preps = {k: prep_chunks(k) for k in range(1, nQB)}
                preps[1][0]()         # block-1 x^T load issued upfront
                for k in range(1, nQB):
                    if k + 1 < nQB:
                        queue.append((k, preps[k + 1][0]))
                    for c in preps[k][1]:
                        queue.append((k, c))
                queue.append((nQB,
                              lambda: nc.sync.dma_start(wpj[:], wp_r)))
                nslots = sum(2 * (n + 1) for n in range(nQB))
                qi = 0
                slot = 0

                def drain(target):
                    nonlocal qi
                    target = int(target)
                    while qi < len(queue) and qi < target:
                        queue[qi][1]()
                        qi += 1

                def drain_deadline(n):
                    nonlocal qi
                    while qi < len(queue) and queue[qi][0] <= n:
                        queue[qi][1]()
                        qi += 1

                # chunks with deadline <= k, for deadline-interpolated pacing
                cumd = [sum(1 for d, _ in queue if d <= k)
                        for k in range(nQB + 1)]

                pending = []
                for n in range(nQB):
                    drain_deadline(n)
                    # full-bank tile so nothing else shares its zero region
                    avs = [ps.tile([128, 512], f32, tag=f"av{h}", bufs=1,
                                   name=f"av{h}_{n}")
                           .rearrange("p (t c) -> p t c", c=128)[:, :, 0:65]
                           for h in (0, 1)]
                    se, av = attention_groups(n, avs)
                    ng = len(se)
                    se[0]()
                    if n == 0:
                        for c in p0v:     # block-0 V pack, after first scores
                            c()
                    for c in pending:
                        c()
                    pending = []
                    slot += 1
                    drain(len(queue) * slot / nslots)
                    for g in range(1, ng):
                        se[g]()
                        av[g - 1]()
                        slot += 1
                        drain(len(queue) * slot / nslots)
                    if n + 1 < nQB:
                        pending = [av[ng - 1],
                                   lambda n=n, avs=avs: normalize(n, avs)]
                    else:
                        # last block: qt0/1 have no contributions from the
                        # final k-tile pair, so their normalize + staging can
                        # overlap the last AV group
                        pending = [
                            lambda n=n, avs=avs: normalize(n, avs, (0,)),
                            lambda n=n, avs=avs: normalize(n, avs, (1,)),
                            av[ng - 1],
                            lambda n=n, avs=avs: normalize(n, avs, (2,)),
                            lambda n=n, avs=avs: normalize(n, avs, (3,)),
                        ]
                for c in pending:
                    c()

            # ---- AllToAll head-shard -> seq-shard -----------------------
            if use_collective:
                nc.gpsimd.collective_compute(
                    "AllToAll", mybir.AluOpType.bypass,
                    ins=[a2a_in.opt()], outs=[a2a_out.opt()],
                    replica_groups=[list(range(N_CORES))])
            else:
                # timing-model variant (TimelineSim can't simulate
                # collectives): stand-in DRAM->DRAM copy
                nc.sync.dma_start(a2a_out[:], a2a_in[:])
            # receive split per proj m-tile so the projection can start as
            # soon as its own q-columns have landed
            a2a_or = a2a_out.rearrange("(j p) q -> p j q", j=8)
            for m in range(SW // 128):
                nc.sync.dma_start(a2a_sb[:, :, 128 * m:128 * (m + 1)],
                                  a2a_or[:, :, 128 * m:128 * (m + 1)])
            if debug:
                nc.sync.dma_start(dbg["qT"][:], qT[:])
                nc.sync.dma_start(dbg["kT"][:], kT[:])
                nc.sync.dma_start(dbg["V2"][:], V2.rearrange("p t c -> p (t c)"))
                nc.sync.dma_start(dbg["ynT"][:], ynT[:])
                nc.sync.dma_start(dbg["a2a"][:], a2a_out[:])

            # ---- output projection for this core's SW rows --------------
            with tc.tile_pool(name="psC", bufs=2, space="PSUM") as psC:
                # keep the PE p-state warm across the collective + receive
                # window so the projection matmuls dispatch at full clock
                wpt = psC.tile([128, QB], f32, tag="warm", bufs=1)
                for i in range(214):
                    nc.tensor.matmul(wpt[:], iden[:], mks[:, 0, :],
                                     start=True, stop=True)
                for i in range(24):
                    nc.tensor.matmul(wpt[:, 0:128], iden[:], mks[:, 0, 0:128],
                                     start=True, stop=True)
                for m in range(SW // 128):
                    pp = psC.tile([128, D_MODEL], f32, tag="pp")
                    for n2 in (0, 1):
                        for k in range(8):
                            nc.tensor.matmul(
                                pp[:, QB * n2:QB * (n2 + 1)],
                                a2a_sb[:, k, 128 * m:128 * (m + 1)],
                                wpj[:, k, QB * n2:QB * (n2 + 1)],
                                start=(k == 0), stop=(k == 7))
                    ob = stg.tile([128, D_MODEL], f32, tag="ob", bufs=2)
                    # split copy+store per 512-col half so the last store
                    # isn't serialized behind a full-row copy
                    for n2 in (0, 1):
                        nc.vector.tensor_copy(ob[:, QB * n2:QB * (n2 + 1)],
                                              pp[:, QB * n2:QB * (n2 + 1)])
                        nc.sync.dma_start(
                            out_d[128 * m:128 * (m + 1),
                                  QB * n2:QB * (n2 + 1)],
                            ob[:, QB * n2:QB * (n2 + 1)])

    if split_waits:
        _split_multi_waits(nc)
    return nc


def make_aux_inputs():
    ident = np.eye(128, dtype=BF16)
    k_idx = np.arange(128)[:, None]
    q_idx = np.arange(QB)[None, :]
    masks = np.stack(
        [((k_idx + 128 * d) <= q_idx).astype(BF16) for d in range(4)], axis=0)
    return ident, masks


def make_in_maps(x, w_qkv, w_proj, seq=SEQ):
    x = np.asarray(x, dtype=np.float32).reshape(seq, D_MODEL)
    xT = np.ascontiguousarray(x.T).astype(BF16)
    w_qkv = np.asarray(w_qkv, dtype=np.float32)
    w_proj = np.ascontiguousarray(w_proj).astype(BF16)
    ident, masks = make_aux_inputs()
    in_maps = []
    for i in range(N_CORES):
        sl = slice(CPC * i, CPC * (i + 1))
        w_slice = np.concatenate(
            [w_qkv[:, sl], w_qkv[:, D_MODEL:][:, sl],
             w_qkv[:, 2 * D_MODEL:][:, sl]], axis=1).astype(BF16)
        in_maps.append({
            "xT": xT,
            "w_slice": np.ascontiguousarray(w_slice),
            "w_proj": w_proj,
            "ident": ident,
            "masks": masks,
        })
    return in_maps


_NC_CACHE = {}


def kernel(x, w_qkv, w_proj):
    """Full inputs in, full output out. Shards internally across 8 cores."""
    try:
        import os
        import jax
        jax.config.update("jax_compilation_cache_dir",
                          os.path.expanduser("~/.cache/jax_bass_kernel"))
        jax.config.update("jax_persistent_cache_min_compile_time_secs", 0.0)
    except Exception:
        pass
    from concourse.bass_utils import run_bass_kernel_spmd

    x = np.asarray(x, dtype=np.float32)
    batch = x.shape[0]
    seq = x.shape[1]
    if seq not in _NC_CACHE:
        _NC_CACHE[seq] = build_nc(seq)
    nc = _NC_CACHE[seq]
    in_maps = make_in_maps(x, w_qkv, w_proj, seq=seq)
    res = run_bass_kernel_spmd(nc, in_maps, list(range(N_CORES)))
    out = np.concatenate([res.results[j]["out"] for j in range(N_CORES)],
                         axis=0)
    return out.reshape(batch, seq, D_MODEL).astype(np.float32)
